# revision 10
# baseline (speedup 1.0000x reference)
"""Trainium2 Bass kernel for a 4-layer compressed model:

    for l in range(4):  x = x @ (base[l] + bitdelta[l] * mask[l])

x: [16, 4096] f32, base/mask: [4, 4096, 4096] f32, bitdelta: [4] f32.

Sharding (8 cores, tensor parallel on weight columns):
  core c owns columns [c*512, (c+1)*512) of every layer's weight.

Key ideas:
  * Low-precision streaming: base is cast to bf16 on the host (quant
    noise ~0.2% of base, itself ~25x smaller than bd*mask); mask is
    +/-1 exactly, which fp8e4m3 represents exactly. Activations ride
    in bf16. HBM traffic per core: 64 MiB (f32) -> 24 MiB. bitdelta
    values are baked into the program as immediates (compiled after
    inputs are known; cache keyed on them).
  * Weight reconstruction is split to balance engines. For half of
    each layer (k-tiles 0..15) the DVE combines W = base + bd*mask
    into bf16 as the chunks land (weight-stream-dependent only, so it
    always runs ahead of the gathers); the PE then needs one matmul
    per k-tile. For the other half, linearity gives
        x @ (base + bd*mask) = x @ base + (bd*x) @ mask,
    so the PE accumulates both raw streams into the same PSUM bank
    using a pre-scaled stationary copy of x (one tiny scaled copy on
    the scalar engine). 48 matmuls/layer total, and the DVE load stays
    at ~21 us so it never gates a layer.
  * Queue discipline: weight DMAs alone ride the sync queue; y^T
    staging, x^T reloads and small copies ride the scalar queue;
    collectives ride gpsimd; combines ride the DVE. Nothing
    gather-dependent ever blocks the weight stream, and 3 layers of
    weights buffer in SBUF.
  * Between layers the [16,512] local result is PE-transposed to
    [512,16] bf16 and AllGather'd on the partition axis into the next
    layer's x^T — exactly the lhsT layout the next matmuls need.
  * PE warmers: a chain of throwaway matmuls anchored on yt_sb (ready
    exactly when the gather is triggered) runs through each gather
    window, holding the HAM clock gate at 2.4 GHz. Cold restarts both
    slow the next layer 2x and skew cores apart, and a collective
    completes only when the slowest rank arrives.

Memory-bound: each core streams 24 MiB of weights; roofline ~70 us.
"""

import numpy as np

import concourse.bass as bass
import concourse.mybir as mybir
import concourse.tile as tile
from concourse import bacc
from concourse.bass_utils import run_bass_kernel_spmd
from concourse.masks import make_identity

L = 4
D = 4096
B = 16
NCORES = 8
C = D // NCORES          # 512 columns per core
KT = D // 128            # 32 contraction tiles of 128
GKB = 8                  # k-tiles per base DMA (1 MiB bf16 transfers)
NGB = KT // GKB          # 4 base DMAs per layer
GKM = 16                 # k-tiles per mask DMA (1 MiB fp8 transfers)
NGM = KT // GKM          # 2 mask DMAs per layer
KHALF = KT // 2          # k-tiles 0..15 combined, 16..31 dual-stream
XCH = 4                  # x^T load chunks per layer
KXC = KT // XCH          # k-tiles per x chunk
CT = C // 128            # 4 transpose chunks
WBUFS_B = 6              # raw base tiles in flight
WBUFS_M = 3              # raw mask tiles in flight
WBUFS_C = 8              # combined tiles (8 MiB: all four layers' halves)
NWARM = 32               # PE-warmer matmuls per gather window

F32 = mybir.dt.float32
BF16 = mybir.dt.bfloat16
FP8 = mybir.dt.float8e4
ALU = mybir.AluOpType
ACT = mybir.ActivationFunctionType

_cache = {}


def build(bd_vals):
    nc = bacc.Bacc(
        "TRN2",
        target_bir_lowering=False,
        debug=False,
        num_devices=NCORES,
    )

    # x^T in natural [4096, 16] order; row d = p*KT + k maps to SBUF
    # partition p, matmul index k — so the load is partition-contiguous.
    xT0 = nc.dram_tensor("xT0", [D, B], BF16, kind="ExternalInput")
    # weight shards, pre-permuted on host: [l, g, p, j*C+c] = W_l[p*KT+g*GK+j,
    # c]; each block is 1 MiB contiguous.
    base_sh = nc.dram_tensor("base_sh", [L, NGB, 128, GKB * C], BF16,
                             kind="ExternalInput")
    mask_sh = nc.dram_tensor("mask_sh", [L, NGM, 128, GKM * C], FP8,
                             kind="ExternalInput")
    out = nc.dram_tensor("out", [B, C], F32, kind="ExternalOutput")

    rg = [list(range(NCORES))]

    def load_xt_chunks(xpool, src):
        """Load x^T [D, B] into 4 SBUF chunk tiles of 8 k-tiles each."""
        chunks = []
        for xc in range(XCH):
            xt = xpool.tile([128, KXC * B], BF16, tag=f"xt{xc}")
            nc.scalar.dma_start(
                xt[:, :].rearrange("p (k b) -> p k b", k=KXC),
                src[:, :].rearrange("(p k) b -> p k b", p=128)
                [:, xc * KXC:(xc + 1) * KXC],
            )
            chunks.append(xt)
        return chunks

    with tile.TileContext(nc) as tc:
        with (
            tc.tile_pool(name="wb", bufs=WBUFS_B) as bpool,
            tc.tile_pool(name="wm", bufs=WBUFS_M) as mpool,
            tc.tile_pool(name="wc", bufs=WBUFS_C) as wcpool,
            tc.tile_pool(name="xp", bufs=2) as xpool,
            tc.tile_pool(name="sp", bufs=2) as spool,
            tc.tile_pool(name="const", bufs=1) as cpool,
            tc.tile_pool(name="acc", bufs=2, space="PSUM") as psum,
            tc.tile_pool(name="tp", bufs=4, space="PSUM") as tpsum,
            tc.tile_pool(name="warm", bufs=1, space="PSUM") as wpsum,
            tc.tile_pool(name="dram", bufs=2, space="DRAM") as dram,
        ):
            ident = cpool.tile([B, B], F32, tag="ident")
            make_identity(nc, ident[:, :])

            xts = load_xt_chunks(xpool, xT0)

            for l in range(L):
                bd = float(bd_vals[l])

                wms = []
                for g in range(NGM):
                    wm = mpool.tile([128, GKM * C], FP8, tag="wm")
                    nc.sync.dma_start(wm[:, :], mask_sh[l, g])
                    wms.append(wm)
                wbs, wcs = [], []
                for g in range(NGB):
                    wb = bpool.tile([128, GKB * C], BF16, tag="wb")
                    nc.sync.dma_start(wb[:, :], base_sh[l, g])
                    wbs.append(wb)
                    if g < NGB // 2:
                        # k-tiles 0..15: combine W = bd*mask + base on DVE.
                        wc = wcpool.tile([128, GKB * C], BF16, tag="wc")
                        half = (g % 2) * (GKB * C)
                        nc.vector.scalar_tensor_tensor(
                            out=wc[:, :],
                            in0=wms[0][:, half:half + GKB * C],
                            scalar=bd,
                            in1=wb[:, :],
                            op0=ALU.mult,
                            op1=ALU.add,
                        )
                        wcs.append(wc)

                # xs = bd * x^T for the dual-stream half (k 16..31).
                xss = {}
                for xc in (2, 3):
                    xs = xpool.tile([128, KXC * B], BF16, tag=f"xs{xc}")
                    nc.scalar.activation(xs[:, :], xts[xc][:, :], ACT.Copy,
                                         scale=bd)
                    xss[xc] = xs

                acc = psum.tile([B, C], F32, tag="acc")
                for k in range(KHALF):
                    xc, kk = k // KXC, k % KXC
                    j = k % GKB
                    nc.tensor.matmul(
                        acc[:, :],
                        xts[xc][:, kk * B:(kk + 1) * B],
                        wcs[k // GKB][:, j * C:(j + 1) * C],
                        start=(k == 0),
                        stop=False,
                    )
                for k in range(KHALF, KT):
                    xc, kk = k // KXC, k % KXC
                    jb, jm = k % GKB, k % GKM
                    nc.tensor.matmul(
                        acc[:, :],
                        xts[xc][:, kk * B:(kk + 1) * B],
                        wbs[k // GKB][:, jb * C:(jb + 1) * C],
                        start=False,
                        stop=False,
                    )
                    nc.tensor.matmul(
                        acc[:, :],
                        xss[xc][:, kk * B:(kk + 1) * B],
                        wms[1][:, jm * C:(jm + 1) * C],
                        start=False,
                        stop=(k == KT - 1),
                    )

                y_sb = spool.tile([B, C], F32, tag="y")
                nc.scalar.copy(y_sb[:, :], acc[:, :])

                if l == L - 1:
                    nc.scalar.dma_start(out[:, :], y_sb[:, :])
                else:
                    # y [16, 512] -> y^T [512, 16] via 4 PE transposes,
                    # then AllGather into the next layer's x^T [4096, 16].
                    yt_sb = spool.tile([128, CT * B], BF16, tag="yt")
                    for cc in range(CT):
                        pt = tpsum.tile([128, B], F32, tag="pt")
                        nc.tensor.transpose(
                            pt[:, :],
                            y_sb[:, cc * 128:(cc + 1) * 128],
                            ident[:, :],
                        )
                        nc.scalar.copy(
                            yt_sb[:, cc * B:(cc + 1) * B], pt[:, :]
                        )
                    ytb = dram.tile([C, B], BF16, tag="ytb")
                    nc.scalar.dma_start(
                        ytb[:, :].rearrange("(cc p) b -> p cc b", p=128),
                        yt_sb[:, :].rearrange("p (cc b) -> p cc b", cc=CT),
                    )
                    xt_full = dram.tile([D, B], BF16, tag="xtf",
                                        addr_space="Shared")
                    nc.gpsimd.collective_compute(
                        "AllGather",
                        ALU.bypass,
                        replica_groups=rg,
                        ins=[ytb.opt()],
                        outs=[xt_full.opt()],
                    )

                    # PE warmers: anchored on yt_sb (ready right at gather
                    # trigger), they run back-to-back through the gather
                    # window on this layer's resident weights.
                    warm_ps = wpsum.tile([B, C], F32, tag="warm")
                    wsrc = wcs[0]
                    for i in range(NWARM):
                        nc.tensor.matmul(
                            warm_ps[:, :],
                            yt_sb[:, :B],
                            wsrc[:, :C],
                            start=(i == 0),
                            stop=(i == NWARM - 1),
                        )

                    xts = load_xt_chunks(xpool, xt_full)

    nc.compile()
    return nc


def _get_nc(bd_vals):
    key = tuple(float(v) for v in bd_vals)
    if _cache.get("key") != key:
        _cache["nc"] = build(bd_vals)
        _cache["key"] = key
    return _cache["nc"]


def _shard_weight(w, gk):
    """[L, D, C] column shard -> [L, KT//gk, 128, gk*C] with
    out[l, g, p, j*C + c] = w[l, p*KT + g*gk + j, c]."""
    ng = KT // gk
    w = w.reshape(L, 128, ng, gk, C)
    w = w.transpose(0, 2, 1, 3, 4)            # [L, ng, 128, gk, C]
    return np.ascontiguousarray(w.reshape(L, ng, 128, gk * C))


def _make_in_maps(x, base, mask, bitdelta):
    import ml_dtypes

    x = np.ascontiguousarray(x, dtype=np.float32)
    base = np.asarray(base, dtype=np.float32)
    mask = np.asarray(mask, dtype=np.float32)

    xT = np.ascontiguousarray(x.T).astype(ml_dtypes.bfloat16)    # [D, B]

    base16 = base.astype(ml_dtypes.bfloat16)
    mask8 = mask.astype(ml_dtypes.float8_e4m3)

    in_maps = []
    for c in range(NCORES):
        sl = slice(c * C, (c + 1) * C)
        in_maps.append({
            "xT0": xT,
            "base_sh": _shard_weight(base16[:, :, sl], GKB),
            "mask_sh": _shard_weight(mask8[:, :, sl], GKM),
        })
    return in_maps


def _run(x, base, mask, bitdelta, trace=False):
    nc = _get_nc(np.asarray(bitdelta, dtype=np.float32))
    in_maps = _make_in_maps(x, base, mask, bitdelta)
    res = run_bass_kernel_spmd(
        nc, in_maps, core_ids=list(range(NCORES)), trace=trace
    )
    y = np.concatenate([res.results[c]["out"] for c in range(NCORES)], axis=1)
    return y, res


def kernel(x, base, mask, bitdelta):
    y, _ = _run(x, base, mask, bitdelta)
    return y
